# revision 21
# baseline (speedup 1.0000x reference)
"""Additive (Bahdanau) attention scores on 8 TRN2 NeuronCores.

scores[b,q,k] = sum_h w[h]*tanh( (queries@U)[b,q,h] + (keys@T)[b,k,h] + bias[h] ) + w_bias

Shapes (hardcoded): B=4, LQ=LK=512, H=128, fp32 in/out.

Sharding: 8 shards = (batch b in 0..3) x (query half in 0..1).
Each core computes its [256 q, 512 k] score block, emitted k-major
([512, 256], scores^T); the host transposes and reassembles.

Per-core device pipeline (partitions = H = 128):
  1. PE (bf16): qU^T = U^T @ queries^T [128h, 256q]; kT^T = T^T @ keys^T [128h, 512k]
  2. DVE: kTb = kT^T + bias (per-partition scalar add, psum->sbuf, bf16 out)
  3. Ramped chunk loop (q-chunk sizes 8,16,24,32...). Queries in a chunk are
     split between two tanh engines:
       ScalarE queries:  DVE tensor_scalar s = kTb + qU[:,q]; one big ACT
                         Tanh over the chunk (exact tanh, 1 elem/lane/cycle).
       VectorE queries:  two chained custom DVE ops compute a clamped deg-7
                         odd polynomial tanh (pass1: z = clamp(s+qU, +-B);
                         pass2: z*(z^2+a)*(z^4+b*z^2+c); the global scale LAM
                         is folded into a second weight vector for the PE
                         reduce). This moves ~10% of tanh work off the
                         ScalarE roofline onto DVE's spare capacity.
       PE: per (q, k-block): score column [128,1] = t_block^T @ w  (FWL bf16)
       DVE: evacuate previous-parity chunk's score columns (+w_bias) -> DMA out
     PSUM score tiles are double-buffered (2 banks per parity) so PE writes
     and DVE evacuation never touch the same bank.
"""

import numpy as np
import ml_dtypes

import concourse.bass as bass
import concourse.bacc as bacc
import concourse.mybir as mybir
import concourse.dve_ops as dops
from concourse.dve_spec import Spec, Src0, C0, C1, C2, Zero, sq, maxx, minn, lower
from concourse.dve_uop import DveOpSpec
from concourse.tile import TileContext

F32 = mybir.dt.float32
BF16 = mybir.dt.bfloat16

B, LQ_FULL, LK, H = 4, 512, 512, 128
N_CORES = 8
LQ = LQ_FULL // 2          # per-core query count (256)

# Ramped chunk sizes (head: first tanh starts early; tail: last PE burst small)
CHUNKS = [8, 16, 24] + [32] * 5 + [24, 16, 8]
assert sum(CHUNKS) == LQ
QMAX = max(CHUNKS)
# queries per chunk handled by the DVE polynomial path (from the chunk tail)
MOVED = [0, 0, 0, 2, 3, 4, 3, 4, 3, 3, 2]
assert len(MOVED) == len(CHUNKS) and all(m < c for m, c in zip(MOVED, CHUNKS))
XMAX = max(MOVED)

# ---- clamped deg-7 odd minimax tanh: tanh(y) ~= LAM*z*(u+PA)*(u^2+PB*u+PC),
#      z = clamp(y, +-B_CLAMP), u = z^2.  Fit: minimax on [0, 2.5].
B_CLAMP = 2.5
_A0, _A1, _A2, _A3 = 0.96904519, -0.23714775, 0.03862184, -0.00247304
LAM = _A3
_rts = np.roots([1.0, _A2 / _A3, _A1 / _A3, _A0 / _A3])
_ra = [r.real for r in _rts if abs(r.imag) < 1e-6][0]
POLY_A = -_ra
POLY_B = (_A2 / _A3) + _ra
POLY_C = -(_A0 / _A3) / _ra


def _ref_clamp(in0, in1, s0, s1, imm2):
    return np.clip(in0.astype(np.float32) + s0, -imm2, imm2)


def _ref_poly7(in0, in1, s0, s1, imm2):
    z = in0.astype(np.float32)
    u = z * z
    return z * (u + s0) * (u * u + s1 * u + imm2)


TANH_CLAMP_ANT = dops.DveOp(
    "TANH_CLAMP_ANT",
    Spec(body=minn(maxx(Src0 + C0, Zero - C2), C2), reference=_ref_clamp),
    subdim=False, uops_sha={},
)
_u = sq(Src0)
TANH_POLY7_ANT = dops.DveOp(
    "TANH_POLY7_ANT",
    Spec(body=Src0 * (_u + C0) * ((sq(_u) + C1 * _u) + C2), reference=_ref_poly7),
    subdim=False, uops_sha={},
)

_REGISTERED = False


def _register_ops():
    global _REGISTERED
    if _REGISTERED:
        return
    for op in (TANH_CLAMP_ANT, TANH_POLY7_ANT):
        if op.name in dops._SUB_OPCODE_FOR_NAME:
            continue
        dops.OPS.append(op)
        row = dops._CUSTOM_DVE_ROW_BASE + len(dops.OPS) - 1
        assert row < 0x20
        dops._SUB_OPCODE_FOR_NAME[op.name] = row
        dops.CUSTOM_DVE_SPECS[op.name] = op.spec
        for ver in ("v3", "v4"):
            try:
                spec_obj = DveOpSpec(name=op.name, opcode=row,
                                     uops=lower(op.spec, ver=ver), rd1_en=False)
                op.uops_sha[ver] = spec_obj.sha(ver)
            except Exception:
                pass
    _REGISTERED = True


_NC_CACHE = None


def _build_nc():
    _register_ops()
    nc = bacc.Bacc()

    # inBF = kT(512) | T(128) | qT(256) | U(128)  bf16  -> [128, 1024]
    #   (k-side first: its DMA lands first and the kT projection starts early)
    # inSC = b | w | wb                             f32   -> [128, 3]
    inBF_d = nc.declare_dram_parameter("inBF", [H, LQ + LK + 2 * H], BF16,
                                       isOutput=False)
    inSC_d = nc.declare_dram_parameter("inSC", [H, 3], F32, isOutput=False)
    out_d = nc.declare_dram_parameter("out", [LK, LQ], F32, isOutput=True)

    with TileContext(nc) as tc:
        with (
            tc.tile_pool(name="const", bufs=1) as cpool,
            tc.tile_pool(name="s", bufs=2) as spool,
            tc.tile_pool(name="z", bufs=2) as zpool,
            tc.tile_pool(name="t", bufs=2) as tpool,
            tc.tile_pool(name="o", bufs=8) as opool,
            tc.tile_pool(name="ps_proj", bufs=1, space="PSUM") as pj_pool,
            tc.tile_pool(name="ps_score", bufs=1, space="PSUM") as sc_pool,
        ):
            # ---- inputs ----
            NBF = LQ + LK + 2 * H
            inSC_sb = cpool.tile([H, 3], F32)
            nc.sync.dma_start(out=inSC_sb, in_=inSC_d[:, :])
            inBF_sb = cpool.tile([H, NBF], BF16)
            half = (LK + H) // 2
            nc.sync.dma_start(out=inBF_sb[:, 0:half], in_=inBF_d[:, 0:half])
            nc.sync.dma_start(out=inBF_sb[:, half:LK + H], in_=inBF_d[:, half:LK + H])
            nc.sync.dma_start(out=inBF_sb[:, LK + H:NBF], in_=inBF_d[:, LK + H:NBF])

            kT_bf = inBF_sb[:, 0:LK]
            T_bf = inBF_sb[:, LK:LK + H]
            qT_bf = inBF_sb[:, LK + H:LK + H + LQ]
            U_bf = inBF_sb[:, LK + H + LQ:NBF]
            b_sb = inSC_sb[:, 0:1]
            w_sb = inSC_sb[:, 1:2]
            wb_sb = inSC_sb[:, 2:3]

            w_bf = cpool.tile([H, 1], BF16)
            nc.vector.tensor_copy(out=w_bf[:, :], in_=w_sb)
            w_lam = cpool.tile([H, 1], F32)
            nc.vector.tensor_scalar_mul(out=w_lam[:, :], in0=w_sb, scalar1=LAM)
            w_lam_bf = cpool.tile([H, 1], BF16)
            nc.vector.tensor_copy(out=w_lam_bf[:, :], in_=w_lam[:, :])

            # ---- projections (bf16 matmuls, fp32 psum) ----
            kTp_ps = pj_pool.tile([H, LK], F32)
            nc.tensor.matmul(kTp_ps[:, :], lhsT=T_bf, rhs=kT_bf,
                             start=True, stop=True)
            kTb_bf = cpool.tile([H, LK], BF16)
            nc.vector.tensor_scalar_add(out=kTb_bf[:, :], in0=kTp_ps[:, :],
                                        scalar1=b_sb)

            qU_ps = pj_pool.tile([H, LQ], F32)
            nc.tensor.matmul(qU_ps[:, :], lhsT=U_bf, rhs=qT_bf,
                             start=True, stop=True)
            qU_sb = cpool.tile([H, LQ], F32)  # fp32: feeds scalar ports
            nc.vector.tensor_copy(out=qU_sb[:, :], in_=qU_ps[:, :])

            # ---- PSUM score accumulators: 2 parities x 2 tiles x [128, 512] ----
            # tile j of a parity holds k-blocks 2j (cols 0:256) and 2j+1 (cols
            # 256:512); column index within a half = absolute q.
            score_ps = [[sc_pool.tile([H, 2 * LQ], F32, name=f"score{p}_{j}")
                         for j in range(2)] for p in range(2)]

            # ---- main loop over ramped chunks ----
            q0 = 0
            ranges = []
            for ci, qc in enumerate(CHUNKS):
                ranges.append((q0, qc))
                par = ci % 2
                x = MOVED[ci]
                a = qc - x  # ScalarE-handled queries (chunk head)
                s_t = spool.tile([H, QMAX * LK], BF16, name="s_t")
                if ci > 0:
                    for qi in range(a):
                        q = q0 + qi
                        nc.vector.tensor_scalar_add(
                            out=s_t[:, qi * LK:(qi + 1) * LK],
                            in0=kTb_bf[:, :],
                            scalar1=qU_sb[:, q:q + 1],
                        )
                z_t = zpool.tile([H, XMAX * LK], F32, name="z_t")
                for xi in range(x):
                    q = q0 + a + xi
                    nc.vector._custom_dve(
                        TANH_CLAMP_ANT,
                        out=z_t[:, xi * LK:(xi + 1) * LK],
                        in0=kTb_bf[:, :],
                        s0=qU_sb[:, q:q + 1], s1=0.0, imm2=B_CLAMP)
                t_t = tpool.tile([H, QMAX * LK], BF16, name="t_t")
                if ci == 0:
                    # fused add+tanh via the ACT bias port: no DVE dependency,
                    # so the ScalarE starts as soon as the projections land.
                    for qi in range(a):
                        q = q0 + qi
                        nc.scalar.activation(t_t[:, qi * LK:(qi + 1) * LK],
                                             kTb_bf[:, :],
                                             mybir.ActivationFunctionType.Tanh,
                                             bias=qU_sb[:, q:q + 1])
                else:
                    nc.scalar.activation(t_t[:, 0:a * LK], s_t[:, 0:a * LK],
                                         mybir.ActivationFunctionType.Tanh)
                for xi in range(x):
                    nc.vector._custom_dve(
                        TANH_POLY7_ANT,
                        out=t_t[:, (a + xi) * LK:(a + xi + 1) * LK],
                        in0=z_t[:, xi * LK:(xi + 1) * LK],
                        s0=POLY_A, s1=POLY_B, imm2=POLY_C)
                if ci >= 1:
                    eq0, eqc = ranges[ci - 1]
                    _evac(nc, opool, score_ps[(ci - 1) % 2], wb_sb, out_d,
                          eq0, eqc, ci - 1)
                for qi in range(qc):
                    q = q0 + qi
                    rhs = w_bf if qi < a else w_lam_bf
                    for kb in range(4):
                        nc.tensor.matmul(
                            score_ps[par][kb // 2][:, (kb % 2) * LQ + q:
                                                   (kb % 2) * LQ + q + 1],
                            lhsT=t_t[:, qi * LK + kb * 128: qi * LK + (kb + 1) * 128],
                            rhs=rhs[:, :],
                            start=True, stop=True,
                        )
                q0 += qc
            # last chunk
            _evac(nc, opool, score_ps[(len(CHUNKS) - 1) % 2], wb_sb, out_d,
                  ranges[-1][0], ranges[-1][1], len(CHUNKS) - 1)

    nc.compile()
    return nc


def _evac(nc, opool, ps_tiles, wb_sb, out_d, q0, qc, ci):
    """Move score columns [q0:q0+qc] of one parity from PSUM to HBM, adding
    w_bias on the way."""
    for kb in range(4):
        o_sb = opool.tile([H, QMAX], F32, name=f"o_sb", tag="o_sb")
        nc.vector.tensor_scalar_add(
            out=o_sb[:, 0:qc],
            in0=ps_tiles[kb // 2][:, (kb % 2) * LQ + q0:(kb % 2) * LQ + q0 + qc],
            scalar1=wb_sb,
        )
        eng = nc.sync if kb % 2 == 0 else nc.gpsimd
        eng.dma_start(out=out_d[kb * 128:(kb + 1) * 128, q0:q0 + qc],
                      in_=o_sb[:, 0:qc])


def get_nc():
    global _NC_CACHE
    if _NC_CACHE is None:
        _NC_CACHE = _build_nc()
    return _NC_CACHE


def make_in_maps(queries, keys, U, T, b, w, w_bias):
    queries = np.asarray(queries, np.float32)
    keys = np.asarray(keys, np.float32)
    U_c = np.asarray(U, np.float32)
    T_c = np.asarray(T, np.float32)
    b_c = np.asarray(b, np.float32).reshape(H, 1)
    w_c = np.asarray(w, np.float32).reshape(H, 1)
    wb_c = np.full((H, 1), np.float32(np.asarray(w_bias)), np.float32)
    inSC = np.ascontiguousarray(np.concatenate([b_c, w_c, wb_c], axis=1))

    in_maps = []
    for core in range(N_CORES):
        bb, qh = core // 2, core % 2
        qT = queries[bb, qh * LQ:(qh + 1) * LQ, :].T
        kT = keys[bb].T
        inBF = np.ascontiguousarray(
            np.concatenate([kT, T_c, qT, U_c], axis=1).astype(ml_dtypes.bfloat16))
        in_maps.append({"inBF": inBF, "inSC": inSC})
    return in_maps


def assemble(results):
    out = np.empty((B, LQ_FULL, LK), np.float32)
    for core in range(N_CORES):
        bb, qh = core // 2, core % 2
        out[bb, qh * LQ:(qh + 1) * LQ, :] = results[core]["out"].T
    return out


def kernel(queries, keys, U, T, b, w, w_bias):
    from concourse.bass_utils import run_bass_kernel_spmd

    nc = get_nc()
    in_maps = make_in_maps(queries, keys, U, T, b, w, w_bias)
    res = run_bass_kernel_spmd(nc, in_maps, core_ids=list(range(N_CORES)))
    return assemble(res.results)


# revision 22
# speedup vs baseline: 1.0044x; 1.0044x over previous
"""Additive (Bahdanau) attention scores on 8 TRN2 NeuronCores.

scores[b,q,k] = sum_h w[h]*tanh( (queries@U)[b,q,h] + (keys@T)[b,k,h] + bias[h] ) + w_bias

Shapes (hardcoded): B=4, LQ=LK=512, H=128, fp32 in/out.

Sharding: 8 shards = (batch b in 0..3) x (query half in 0..1).
Each core computes its [256 q, 512 k] score block, emitted k-major
([512, 256], scores^T); the host transposes and reassembles.

Per-core device pipeline (partitions = H = 128):
  1. PE (bf16): qU^T = U^T @ queries^T [128h, 256q]; kT^T = T^T @ keys^T [128h, 512k]
  2. DVE: kTb = kT^T + bias (per-partition scalar add, psum->sbuf, bf16 out)
  3. Ramped chunk loop (q-chunk sizes 8,16,24,32...). Queries in a chunk are
     split between two tanh engines:
       ScalarE queries:  DVE tensor_scalar s = kTb + qU[:,q]; one big ACT
                         Tanh over the chunk (exact tanh, 1 elem/lane/cycle).
       VectorE queries:  two chained custom DVE ops compute a clamped deg-7
                         odd polynomial tanh (pass1: z = clamp(s+qU, +-B);
                         pass2: z*(z^2+a)*(z^4+b*z^2+c); the global scale LAM
                         is folded into a second weight vector for the PE
                         reduce). This moves ~10% of tanh work off the
                         ScalarE roofline onto DVE's spare capacity.
       PE: per (q, k-block): score column [128,1] = t_block^T @ w  (FWL bf16)
       DVE: evacuate previous-parity chunk's score columns (+w_bias) -> DMA out
     PSUM score tiles are double-buffered (2 banks per parity) so PE writes
     and DVE evacuation never touch the same bank.
"""

import numpy as np
import ml_dtypes

import concourse.bass as bass
import concourse.bacc as bacc
import concourse.mybir as mybir
import concourse.dve_ops as dops
from concourse.dve_spec import Spec, Src0, C0, C1, C2, Zero, sq, maxx, minn, lower
from concourse.dve_uop import DveOpSpec
from concourse.tile import TileContext

F32 = mybir.dt.float32
BF16 = mybir.dt.bfloat16

B, LQ_FULL, LK, H = 4, 512, 512, 128
N_CORES = 8
LQ = LQ_FULL // 2          # per-core query count (256)

# Ramped chunk sizes (head: first tanh starts early; tail: last PE burst small)
CHUNKS = [8, 16, 24] + [32] * 5 + [24, 16, 8]
assert sum(CHUNKS) == LQ
QMAX = max(CHUNKS)
# queries per chunk handled by the DVE polynomial path (from the chunk tail)
MOVED = [0, 0, 0, 2, 3, 3, 3, 3, 3, 3, 2]
assert len(MOVED) == len(CHUNKS) and all(m < c for m, c in zip(MOVED, CHUNKS))
XMAX = max(MOVED)

# ---- clamped deg-7 odd minimax tanh: tanh(y) ~= LAM*z*(u+PA)*(u^2+PB*u+PC),
#      z = clamp(y, +-B_CLAMP), u = z^2.  Fit: minimax on [0, 2.5].
B_CLAMP = 2.5
_A0, _A1, _A2, _A3 = 0.96904519, -0.23714775, 0.03862184, -0.00247304
LAM = _A3
_rts = np.roots([1.0, _A2 / _A3, _A1 / _A3, _A0 / _A3])
_ra = [r.real for r in _rts if abs(r.imag) < 1e-6][0]
POLY_A = -_ra
POLY_B = (_A2 / _A3) + _ra
POLY_C = -(_A0 / _A3) / _ra


def _ref_clamp(in0, in1, s0, s1, imm2):
    return np.clip(in0.astype(np.float32) + s0, -imm2, imm2)


def _ref_poly7(in0, in1, s0, s1, imm2):
    z = in0.astype(np.float32)
    u = z * z
    return z * (u + s0) * (u * u + s1 * u + imm2)


TANH_CLAMP_ANT = dops.DveOp(
    "TANH_CLAMP_ANT",
    Spec(body=minn(maxx(Src0 + C0, Zero - C2), C2), reference=_ref_clamp),
    subdim=False, uops_sha={},
)
_u = sq(Src0)
TANH_POLY7_ANT = dops.DveOp(
    "TANH_POLY7_ANT",
    Spec(body=Src0 * (_u + C0) * ((sq(_u) + C1 * _u) + C2), reference=_ref_poly7),
    subdim=False, uops_sha={},
)

_REGISTERED = False


def _register_ops():
    global _REGISTERED
    if _REGISTERED:
        return
    for op in (TANH_CLAMP_ANT, TANH_POLY7_ANT):
        if op.name in dops._SUB_OPCODE_FOR_NAME:
            continue
        dops.OPS.append(op)
        row = dops._CUSTOM_DVE_ROW_BASE + len(dops.OPS) - 1
        assert row < 0x20
        dops._SUB_OPCODE_FOR_NAME[op.name] = row
        dops.CUSTOM_DVE_SPECS[op.name] = op.spec
        for ver in ("v3", "v4"):
            try:
                spec_obj = DveOpSpec(name=op.name, opcode=row,
                                     uops=lower(op.spec, ver=ver), rd1_en=False)
                op.uops_sha[ver] = spec_obj.sha(ver)
            except Exception:
                pass
    _REGISTERED = True


_NC_CACHE = None


def _build_nc():
    _register_ops()
    nc = bacc.Bacc()

    # inBF = kT(512) | T(128) | qT(256) | U(128)  bf16  -> [128, 1024]
    #   (k-side first: its DMA lands first and the kT projection starts early)
    # inSC = b | w | wb                             f32   -> [128, 3]
    inBF_d = nc.declare_dram_parameter("inBF", [H, LQ + LK + 2 * H], BF16,
                                       isOutput=False)
    inSC_d = nc.declare_dram_parameter("inSC", [H, 3], F32, isOutput=False)
    out_d = nc.declare_dram_parameter("out", [LK, LQ], F32, isOutput=True)

    with TileContext(nc) as tc:
        with (
            tc.tile_pool(name="const", bufs=1) as cpool,
            tc.tile_pool(name="s", bufs=2) as spool,
            tc.tile_pool(name="z", bufs=2) as zpool,
            tc.tile_pool(name="t", bufs=2) as tpool,
            tc.tile_pool(name="o", bufs=8) as opool,
            tc.tile_pool(name="ps_proj", bufs=1, space="PSUM") as pj_pool,
            tc.tile_pool(name="ps_score", bufs=1, space="PSUM") as sc_pool,
        ):
            # ---- inputs ----
            NBF = LQ + LK + 2 * H
            inSC_sb = cpool.tile([H, 3], F32)
            nc.sync.dma_start(out=inSC_sb, in_=inSC_d[:, :])
            inBF_sb = cpool.tile([H, NBF], BF16)
            half = (LK + H) // 2
            nc.sync.dma_start(out=inBF_sb[:, 0:half], in_=inBF_d[:, 0:half])
            nc.sync.dma_start(out=inBF_sb[:, half:LK + H], in_=inBF_d[:, half:LK + H])
            nc.sync.dma_start(out=inBF_sb[:, LK + H:NBF], in_=inBF_d[:, LK + H:NBF])

            kT_bf = inBF_sb[:, 0:LK]
            T_bf = inBF_sb[:, LK:LK + H]
            qT_bf = inBF_sb[:, LK + H:LK + H + LQ]
            U_bf = inBF_sb[:, LK + H + LQ:NBF]
            b_sb = inSC_sb[:, 0:1]
            w_sb = inSC_sb[:, 1:2]
            wb_sb = inSC_sb[:, 2:3]

            w_bf = cpool.tile([H, 1], BF16)
            nc.vector.tensor_copy(out=w_bf[:, :], in_=w_sb)
            w_lam = cpool.tile([H, 1], F32)
            nc.vector.tensor_scalar_mul(out=w_lam[:, :], in0=w_sb, scalar1=LAM)
            w_lam_bf = cpool.tile([H, 1], BF16)
            nc.vector.tensor_copy(out=w_lam_bf[:, :], in_=w_lam[:, :])

            # ---- projections (bf16 matmuls, fp32 psum) ----
            kTp_ps = pj_pool.tile([H, LK], F32)
            nc.tensor.matmul(kTp_ps[:, :], lhsT=T_bf, rhs=kT_bf,
                             start=True, stop=True)
            kTb_bf = cpool.tile([H, LK], BF16)
            nc.vector.tensor_scalar_add(out=kTb_bf[:, :], in0=kTp_ps[:, :],
                                        scalar1=b_sb)

            qU_ps = pj_pool.tile([H, LQ], F32)
            nc.tensor.matmul(qU_ps[:, :], lhsT=U_bf, rhs=qT_bf,
                             start=True, stop=True)
            qU_sb = cpool.tile([H, LQ], F32)  # fp32: feeds scalar ports
            nc.vector.tensor_copy(out=qU_sb[:, :], in_=qU_ps[:, :])

            # ---- PSUM score accumulators: 2 parities x 2 tiles x [128, 512] ----
            # tile j of a parity holds k-blocks 2j (cols 0:256) and 2j+1 (cols
            # 256:512); column index within a half = absolute q.
            score_ps = [[sc_pool.tile([H, 2 * LQ], F32, name=f"score{p}_{j}")
                         for j in range(2)] for p in range(2)]

            # ---- main loop over ramped chunks ----
            q0 = 0
            ranges = []
            for ci, qc in enumerate(CHUNKS):
                ranges.append((q0, qc))
                par = ci % 2
                x = MOVED[ci]
                a = qc - x  # ScalarE-handled queries (chunk head)
                s_t = spool.tile([H, QMAX * LK], BF16, name="s_t")
                if ci > 0:
                    for qi in range(a):
                        q = q0 + qi
                        nc.vector.tensor_scalar_add(
                            out=s_t[:, qi * LK:(qi + 1) * LK],
                            in0=kTb_bf[:, :],
                            scalar1=qU_sb[:, q:q + 1],
                        )
                z_t = zpool.tile([H, XMAX * LK], F32, name="z_t")
                for xi in range(x):
                    q = q0 + a + xi
                    nc.vector._custom_dve(
                        TANH_CLAMP_ANT,
                        out=z_t[:, xi * LK:(xi + 1) * LK],
                        in0=kTb_bf[:, :],
                        s0=qU_sb[:, q:q + 1], s1=0.0, imm2=B_CLAMP)
                t_t = tpool.tile([H, QMAX * LK], BF16, name="t_t")
                if ci == 0:
                    # fused add+tanh via the ACT bias port: no DVE dependency,
                    # so the ScalarE starts as soon as the projections land.
                    for qi in range(a):
                        q = q0 + qi
                        nc.scalar.activation(t_t[:, qi * LK:(qi + 1) * LK],
                                             kTb_bf[:, :],
                                             mybir.ActivationFunctionType.Tanh,
                                             bias=qU_sb[:, q:q + 1])
                else:
                    nc.scalar.activation(t_t[:, 0:a * LK], s_t[:, 0:a * LK],
                                         mybir.ActivationFunctionType.Tanh)
                for xi in range(x):
                    nc.vector._custom_dve(
                        TANH_POLY7_ANT,
                        out=t_t[:, (a + xi) * LK:(a + xi + 1) * LK],
                        in0=z_t[:, xi * LK:(xi + 1) * LK],
                        s0=POLY_A, s1=POLY_B, imm2=POLY_C)
                if ci >= 1:
                    eq0, eqc = ranges[ci - 1]
                    _evac(nc, opool, score_ps[(ci - 1) % 2], wb_sb, out_d,
                          eq0, eqc, ci - 1)
                for qi in range(qc):
                    q = q0 + qi
                    rhs = w_bf if qi < a else w_lam_bf
                    for kb in range(4):
                        nc.tensor.matmul(
                            score_ps[par][kb // 2][:, (kb % 2) * LQ + q:
                                                   (kb % 2) * LQ + q + 1],
                            lhsT=t_t[:, qi * LK + kb * 128: qi * LK + (kb + 1) * 128],
                            rhs=rhs[:, :],
                            start=True, stop=True,
                        )
                q0 += qc
            # last chunk
            _evac(nc, opool, score_ps[(len(CHUNKS) - 1) % 2], wb_sb, out_d,
                  ranges[-1][0], ranges[-1][1], len(CHUNKS) - 1)

    nc.compile()
    return nc


def _evac(nc, opool, ps_tiles, wb_sb, out_d, q0, qc, ci):
    """Move score columns [q0:q0+qc] of one parity from PSUM to HBM, adding
    w_bias on the way."""
    for kb in range(4):
        o_sb = opool.tile([H, QMAX], F32, name=f"o_sb", tag="o_sb")
        nc.vector.tensor_scalar_add(
            out=o_sb[:, 0:qc],
            in0=ps_tiles[kb // 2][:, (kb % 2) * LQ + q0:(kb % 2) * LQ + q0 + qc],
            scalar1=wb_sb,
        )
        eng = nc.sync if kb % 2 == 0 else nc.gpsimd
        eng.dma_start(out=out_d[kb * 128:(kb + 1) * 128, q0:q0 + qc],
                      in_=o_sb[:, 0:qc])


def get_nc():
    global _NC_CACHE
    if _NC_CACHE is None:
        _NC_CACHE = _build_nc()
    return _NC_CACHE


def make_in_maps(queries, keys, U, T, b, w, w_bias):
    queries = np.asarray(queries, np.float32)
    keys = np.asarray(keys, np.float32)
    U_c = np.asarray(U, np.float32)
    T_c = np.asarray(T, np.float32)
    b_c = np.asarray(b, np.float32).reshape(H, 1)
    w_c = np.asarray(w, np.float32).reshape(H, 1)
    wb_c = np.full((H, 1), np.float32(np.asarray(w_bias)), np.float32)
    inSC = np.ascontiguousarray(np.concatenate([b_c, w_c, wb_c], axis=1))

    in_maps = []
    for core in range(N_CORES):
        bb, qh = core // 2, core % 2
        qT = queries[bb, qh * LQ:(qh + 1) * LQ, :].T
        kT = keys[bb].T
        inBF = np.ascontiguousarray(
            np.concatenate([kT, T_c, qT, U_c], axis=1).astype(ml_dtypes.bfloat16))
        in_maps.append({"inBF": inBF, "inSC": inSC})
    return in_maps


def assemble(results):
    out = np.empty((B, LQ_FULL, LK), np.float32)
    for core in range(N_CORES):
        bb, qh = core // 2, core % 2
        out[bb, qh * LQ:(qh + 1) * LQ, :] = results[core]["out"].T
    return out


def kernel(queries, keys, U, T, b, w, w_bias):
    from concourse.bass_utils import run_bass_kernel_spmd

    nc = get_nc()
    in_maps = make_in_maps(queries, keys, U, T, b, w, w_bias)
    res = run_bass_kernel_spmd(nc, in_maps, core_ids=list(range(N_CORES)))
    return assemble(res.results)
